# revision 37
# baseline (speedup 1.0000x reference)
"""GQA attention (16 q heads / 4 kv heads, HD=128, S=4096, D=2048) with RoPE,
causal mask, log-gate on kv positions, softmax, and output projection —
distributed over 8 NeuronCores.

Sharding: head-parallel. Core c computes q heads {2c, 2c+1} and kv head c//2.
Wq/Wk/Wv are split column-wise, Wo row-wise; each core produces a partial
[S, D] (bf16) output and the host sums the 8 partials in fp32.

On-device strategy:
 - Attention computed transposed (scores^T [j, i]) so the attnV matmul needs
   no weight transposes; kv gate folded into exp via per-partition bias:
   ex = exp(s + log g_j). Both heads are processed together per key block —
   scores land in one 2-bank [128,1024] PSUM tile so a single exp ACT and a
   single esum add cover both heads, and the softmax-denominator tail
   (rowsum matmul -> 1/x = exp(-ln x) on Scalar -> GPSIMD partition
   broadcast) runs once per chunk. Exp and Ln are steered into the combined
   natural_log_exp_and_others activation table so no table reloads occur.
 - Causality: upper-triangle blocks skipped, diagonal blocks width-trimmed
   (only i >= jb*128 columns computed), boundary 128x128 triangle zeroed by a
   bf16 0/1 mask multiply on ex. Max-free softmax (scores bounded ~[-6,7]).
 - RoPE in bf16 (tabs stored bf16), bf16 PE transposes for V, output
   partials in bf16 (summed fp32 on host).
 - Emission is software-pipelined for the in-order engine queues: a PE
   warmup (HAM), attnV/esum consumption lagging scores by 2 blocks, and Wo
   chunks of the previous sequence chunk interleaved into the attention loop
   keep the Tensor engine >85% busy.
"""

import math
from contextlib import ExitStack

import numpy as np

import concourse.bass as bass
import concourse.mybir as mybir
import concourse.tile as tile
from concourse import bacc
from concourse._compat import with_exitstack
from concourse.bass import ds
from concourse.bass_utils import run_bass_kernel_spmd
from concourse.masks import make_identity

P = 128
F = 512            # free-dim chunk (one PSUM bank of fp32)
S = 4096
D = 2048
HD = 128
KO = D // P        # 16 k-chunks for the projections
NB = S // F        # 8 sequence chunks
NJB = S // P       # 32 key blocks
F32 = mybir.dt.float32
BF16 = mybir.dt.bfloat16


@with_exitstack
def _body(ctx: ExitStack, tc: tile.TileContext, io: dict):
    nc = tc.nc

    persist = ctx.enter_context(tc.tile_pool(name="persist", bufs=1))
    psProj = ctx.enter_context(tc.tile_pool(name="psProj", bufs=2, space="PSUM"))
    junk = persist.tile([P, P], BF16, tag="junk")
    # PE warmup FIRST, gated only on a cheap memset (no DMA dependencies;
    # outputs never read): keeps the HAM activity window busy from t~0
    # during the initial DMAs so the first real matmuls run at full clock.
    nc.gpsimd.memset(junk[:], 0.0)
    ps_w = psProj.tile([P, F], F32, tag="ps", name="warm")
    for wi in range(44):
        nc.tensor.matmul(ps_w[:, :P], lhsT=junk[:], rhs=junk[:],
                         start=True, stop=True)
    qT = persist.tile([P, 2, S], BF16, tag="qT")        # [d, h, i]
    kT = persist.tile([P, S], BF16, tag="kT")           # [d, j]
    vv = persist.tile([P, NJB * HD], BF16, tag="vv")    # [j, jb*d] flat
    attnT = persist.tile([P, 2, S], BF16, tag="attnT")  # [d, h, i] normalized
    glog = persist.tile([P, NJB], F32, tag="glog")      # log-gate columns
    trimask = persist.tile([P, P], BF16, tag="trimask")
    ident = persist.tile([P, P], F32, tag="ident")
    identb = persist.tile([P, P], BF16, tag="identb")
    onescol = persist.tile([P, 1], BF16, tag="onescol")

    make_identity(nc, ident[:])
    nc.vector.tensor_copy(identb[:], ident[:])
    nc.gpsimd.memset(onescol[:], 1.0)

    xt_r = io["xt"].rearrange("(ko p) s -> p ko s", p=P)  # [128, 16, 4096]

    wpool = ctx.enter_context(tc.tile_pool(name="wpool", bufs=1))
    xt_pool = ctx.enter_context(tc.tile_pool(name="xt", bufs=6))
    tab_pool = ctx.enter_context(tc.tile_pool(name="tab", bufs=2))
    rope_pool = ctx.enter_context(tc.tile_pool(name="rope", bufs=4))
    exp_pool = ctx.enter_context(tc.tile_pool(name="exp", bufs=6))
    esum_pool = ctx.enter_context(tc.tile_pool(name="esum", bufs=2))
    bc_pool = ctx.enter_context(tc.tile_pool(name="bc", bufs=3))
    ob_pool = ctx.enter_context(tc.tile_pool(name="ob", bufs=2))
    psSc = ctx.enter_context(tc.tile_pool(name="psSc", bufs=2, space="PSUM"))
    psOut = ctx.enter_context(tc.tile_pool(name="psOut", bufs=1, space="PSUM"))

    # DMA order tuned for startup latency: the k-projection (first consumer)
    # needs wk + the first x chunk; heavier weights follow. Weight DMAs are
    # split into per-ko-group transfers so they spread across DMA queues and
    # the first chain can start before the whole tile has landed.
    wq = wpool.tile([P, KO, 2 * HD], BF16, tag="wq")
    wk = wpool.tile([P, KO, HD], BF16, tag="wk")
    wv = wpool.tile([P, KO, HD], BF16, tag="wv")
    wo = wpool.tile([P, 2, D], BF16, tag="wo")
    wk_r = io["wk"].rearrange("(ko p) m -> p ko m", p=P)
    for kg in range(4):
        nc.sync.dma_start(wk[:, ds(kg * 4, 4), :], wk_r[:, ds(kg * 4, 4), :])

    def late_dmas():
        wq_r = io["wq"].rearrange("(ko p) m -> p ko m", p=P)
        for kg in range(4):
            nc.sync.dma_start(wq[:, ds(kg * 4, 4), :], wq_r[:, ds(kg * 4, 4), :])
        nc.sync.dma_start(wv[:], io["wv"].rearrange("(ko p) m -> p ko m", p=P))
        nc.sync.dma_start(glog[:], io["glog"])
        nc.sync.dma_start(trimask[:], io["trimask"])
        nc.sync.dma_start(wo[:], io["wo"].rearrange("(h p) e -> p h e", p=P))

    # ---- emission helpers -------------------------------------------------

    def emit_proj(nb, after_dmas=None):
        sl = ds(nb * F, F)
        xq = []
        for xi in range(4):
            xtile = xt_pool.tile([P, 4, F], BF16, tag="xt")
            if nb == 0 and xi == 0:
                # Finer DMA granularity so the very first chain matmul only
                # waits for its own ko slice, not the whole tile.
                for ko in range(4):
                    nc.sync.dma_start(
                        xtile[:, ko, :], xt_r[:, xi * 4 + ko, sl]
                    )
            else:
                nc.sync.dma_start(xtile[:], xt_r[:, ds(xi * 4, 4), sl])
            xq.append(xtile)
        tabs = tab_pool.tile([P, 4, F], BF16, tag="tabs")
        nc.sync.dma_start(tabs[:], io["tabs"][:, :, sl])
        if after_dmas is not None:
            # Deferred init DMAs must be EMITTED before any instruction that
            # reads them — Tile dependency tracking is emission-order based.
            after_dmas()

        def proj_rope(w_sb, m0, ct, st, dest):
            ps = psProj.tile([P, F], F32, tag="ps")
            for ko in range(KO):
                nc.tensor.matmul(
                    ps[:],
                    lhsT=w_sb[:, ko, ds(m0, P)],
                    rhs=xq[ko // 4][:, ko % 4, :],
                    start=(ko == 0),
                    stop=(ko == KO - 1),
                )
            tmp = rope_pool.tile([P, F], BF16, tag="tmp")
            nc.scalar.copy(tmp[:], ps[:])
            rot = rope_pool.tile([P, F], BF16, tag="rot")
            nc.sync.dma_start(rot[0:64, :], tmp[64:128, :])
            nc.sync.dma_start(rot[64:128, :], tmp[0:64, :])
            r2 = rope_pool.tile([P, F], BF16, tag="r2")
            nc.vector.tensor_tensor(r2[:], rot[:], st, op=mybir.AluOpType.mult)
            t1 = rope_pool.tile([P, F], BF16, tag="t1")
            nc.vector.tensor_tensor(t1[:], tmp[:], ct, op=mybir.AluOpType.mult)
            nc.vector.tensor_tensor(dest, t1[:], r2[:], op=mybir.AluOpType.add)

        proj_rope(wk, 0, tabs[:, 2, :], tabs[:, 3, :], kT[:, sl])
        proj_rope(wq, 0, tabs[:, 0, :], tabs[:, 1, :], qT[:, 0, sl])
        proj_rope(wq, P, tabs[:, 0, :], tabs[:, 1, :], qT[:, 1, sl])

        psv = psProj.tile([P, F], F32, tag="ps")
        for ko in range(KO):
            nc.tensor.matmul(
                psv[:],
                lhsT=wv[:, ko, :],
                rhs=xq[ko // 4][:, ko % 4, :],
                start=(ko == 0),
                stop=(ko == KO - 1),
            )
        vT = rope_pool.tile([P, F], BF16, tag="vT")
        nc.scalar.copy(vT[:], psv[:])
        pt = psProj.tile([P, F], BF16, tag="ps", name=f"pt_{nb}")
        for isub in range(4):
            nc.tensor.transpose(
                pt[:, ds(isub * P, P)], vT[:, ds(isub * P, P)], identb[:]
            )
        nc.scalar.copy(vv[:, ds(nb * 4 * HD, 4 * HD)], pt[:])

    def make_wo_emitters(nb, pools=None, mixed_copies=False):
        """Generator of per-chunk Wo emitters for sequence chunk nb. The
        default pool is psProj: the projection phase and the Wo/attention
        phase of adjacent chunks don't overlap, so Wo chunks rotate through
        the same two banks. The trailing call passes both free pools so
        copies of consecutive chunks overlap."""
        pools = pools or [psProj]
        emitters = []
        cnt = [0]
        for i4 in range(4):
            i2 = nb * 4 + i4
            ob = ob_pool.tile([P, D], BF16, tag="ob", name=f"ob_{i2}")
            for e in range(4):
                def emit(i2=i2, e=e, ob=ob):
                    pool = pools[cnt[0] % len(pools)]
                    cnt[0] += 1
                    po = pool.tile([P, F], F32, tag="sc" if pool is psSc else "ps",
                                   name=f"po_{i2}_{e}")
                    for h in range(2):
                        nc.tensor.matmul(
                            po[:],
                            lhsT=attnT[:, h, ds(i2 * P, P)],
                            rhs=wo[:, h, ds(e * F, F)],
                            start=(h == 0),
                            stop=(h == 1),
                        )
                    if (e % 2 == 0) if mixed_copies else (e == 0):
                        nc.scalar.copy(ob[:, ds(e * F, F)], po[:])
                    else:
                        nc.vector.tensor_copy(ob[:, ds(e * F, F)], po[:])
                    if e % 2 == 1:
                        # DMA out in halves so the first half overlaps the
                        # remaining copies (shortens the kernel end tail).
                        def dma(i2=i2, e=e, ob=ob):
                            half = ds((e - 1) * F, 2 * F)
                            nc.sync.dma_start(
                                io["outp"][ds(i2 * P, P), half], ob[:, half]
                            )
                        return dma
                    return None
                emitters.append(emit)
        return emitters

    def emit_attention(nb, wo_emitters):
        """Attention for q-chunk nb, both heads processed TOGETHER per key
        block: scores land in one 2-bank [P, 2F] PSUM tile ([h0 | h1]), so a
        single exp ACT covers both heads (same jb => same gate bias), and a
        single DVE add accumulates both exp-sums. Wo chunk emitters from the
        previous nb interleave to keep the PE queue dense; attnV/esum
        consumption lags scores by 2 blocks so the in-order PE queue never
        waits on the Scalar engine."""
        sl = ds(nb * F, F)
        njb = 4 * nb + 4
        wo_idx = 0
        pending_dma = []
        WO_DELAY = 3

        def emit_wo_share(it):
            nonlocal wo_idx
            if it < WO_DELAY:
                return
            want = ((it + 1 - WO_DELAY) * len(wo_emitters)) // (njb - WO_DELAY)
            while wo_idx < want:
                d = wo_emitters[wo_idx]()
                if d is not None:
                    pending_dma.append(d)
                wo_idx += 1

        out_h = [
            psOut.tile([P, F], F32, tag=f"o{h}", name=f"out_{nb}_{h}")
            for h in range(2)
        ]
        esum = esum_pool.tile([P, 2 * F], BF16, tag="esum", name=f"es_{nb}")
        pending = []  # [(jb, ex, trim, width)] lag queue

        def consume():
            pjb, pex, ptrim, pwidth = pending.pop(0)
            for h in range(2):
                nc.tensor.matmul(
                    out_h[h][:, ds(ptrim, pwidth)],
                    lhsT=vv[:, ds(pjb * HD, HD)],
                    rhs=pex[:, ds(h * F, pwidth)],
                    start=(pjb == 0),
                    stop=(pjb == njb - 1),
                )
            if pjb == 0:
                nc.vector.tensor_copy(esum[:], pex[:])
            elif pwidth == F:
                nc.vector.tensor_tensor(
                    esum[:], esum[:], pex[:], op=mybir.AluOpType.add
                )
            else:
                for h in range(2):
                    nc.vector.tensor_tensor(
                        esum[:, ds(h * F + ptrim, pwidth)],
                        esum[:, ds(h * F + ptrim, pwidth)],
                        pex[:, ds(h * F, pwidth)],
                        op=mybir.AluOpType.add,
                    )

        for jb in range(njb):
            dp = jb - 4 * nb
            trim = max(0, dp) * P
            width = F - trim
            sc_ps = psSc.tile([P, 2 * F], F32, tag="sc")
            for h in range(2):
                nc.tensor.matmul(
                    sc_ps[:, ds(h * F + trim, width)],
                    lhsT=kT[:, ds(jb * P, P)],
                    rhs=qT[:, h, ds(nb * F + trim, width)],
                    start=True,
                    stop=True,
                )
            ex = exp_pool.tile([P, 2 * F], BF16, tag="ex")
            if width == F:
                nc.scalar.activation(
                    ex[:], sc_ps[:],
                    mybir.ActivationFunctionType.Exp,
                    bias=glog[:, jb : jb + 1],
                )
            else:
                for h in range(2):
                    nc.scalar.activation(
                        ex[:, ds(h * F, width)],
                        sc_ps[:, ds(h * F + trim, width)],
                        mybir.ActivationFunctionType.Exp,
                        bias=glog[:, jb : jb + 1],
                    )
            if dp >= 0:
                for h in range(2):
                    nc.vector.tensor_tensor(
                        ex[:, ds(h * F, P)], ex[:, ds(h * F, P)], trimask[:],
                        op=mybir.AluOpType.mult,
                    )
            emit_wo_share(jb)
            pending.append((jb, ex, trim, width))
            if len(pending) > 2:
                consume()
        while pending:
            consume()
        # denominator for both heads: M=1 rowsum matmuls, then 1/x =
        # exp(-ln(x)) on the Scalar engine (DVE reciprocal is 8 cyc/elem —
        # too slow for the critical tail), broadcast across partitions.
        # Per-head tail so Scalar/GPSIMD/DVE stages pipeline and the first
        # Wo matmul of the next phase starts ~2us earlier (keeps the PE idle
        # window under the HAM re-throttle threshold on the last chunk).
        den_ps = psSc.tile([1, 2 * F], F32, tag="sc", name=f"den_{nb}")
        for h in range(2):
            nc.tensor.matmul(
                den_ps[0:1, ds(h * F, F)], lhsT=onescol[:],
                rhs=esum[:, ds(h * F, F)], start=True, stop=True,
            )
        for h in range(2):
            lrow = bc_pool.tile([1, F], F32, tag="lrow", name=f"lr_{nb}_{h}")
            nc.scalar.activation(
                lrow[:], den_ps[0:1, ds(h * F, F)],
                mybir.ActivationFunctionType.Ln,
            )
            rrow = bc_pool.tile([1, F], F32, tag="rrow", name=f"rr_{nb}_{h}")
            nc.scalar.activation(
                rrow[:], lrow[:], mybir.ActivationFunctionType.Exp, scale=-1.0
            )
            rbc = bc_pool.tile([P, F], F32, tag="rbc", name=f"rb_{nb}_{h}")
            nc.gpsimd.partition_broadcast(rbc[:], rrow[0:1, :])
            nc.vector.tensor_tensor(
                attnT[:, h, sl], out_h[h][:], rbc[:],
                op=mybir.AluOpType.mult,
            )
        while wo_idx < len(wo_emitters):
            d = wo_emitters[wo_idx]()
            if d is not None:
                pending_dma.append(d)
            wo_idx += 1
        for d in pending_dma:
            d()

    # ---- main program -----------------------------------------------------
    for nb in range(NB):
        emit_proj(nb, after_dmas=late_dmas if nb == 0 else None)
        wo_emitters = make_wo_emitters(nb - 1) if nb > 0 else []
        emit_attention(nb, wo_emitters)
    # trailing Wo for the last chunk: use the (now idle) score pool's banks
    # so consecutive chunks pipeline, and split copies across both engines.
    for em in make_wo_emitters(NB - 1, pools=[psSc, psProj], mixed_copies=True):
        d = em()
        if d is not None:
            d()


_NC_CACHE = None


def _prefer_combined_exp_ln_table():
    """This kernel uses Exp (attention) and Ln (softmax reciprocal) — the
    default greedy table-set picker alternates exp_and_others/natural_log,
    inserting ~33 ACT_TABLE_LOADs (~1.3us each, on the critical tail).
    Claim Exp/Ln live only in natural_log_exp_and_others so a single set is
    loaded once. Set indices (act_func_set_id = position in act_info.json)
    are preserved; only compile-time table *selection* changes."""
    import functools
    from concourse import bacc as _bacc

    orig = _bacc.get_activation_tables

    @functools.cache
    def patched(arch):
        tabs = orig(arch)
        exp = mybir.ActivationFunctionType.Exp
        ln = mybir.ActivationFunctionType.Ln
        if not any(
            exp in fns and ln in fns for fns in tabs.values()
        ):
            return tabs
        out = {}
        for name, fns in tabs.items():
            if exp in fns and ln in fns:
                out[name] = fns
            else:
                out[name] = fns - {exp, ln}
        return out

    _bacc.get_activation_tables = patched


def build_nc():
    global _NC_CACHE
    if _NC_CACHE is not None:
        return _NC_CACHE
    _prefer_combined_exp_ln_table()
    nc = bacc.Bacc("TRN2", target_bir_lowering=False, debug=False)
    io = {
        "xt": nc.dram_tensor("xt", [D, S], BF16, kind="ExternalInput").ap(),
        "wq": nc.dram_tensor("wq", [D, 2 * HD], BF16, kind="ExternalInput").ap(),
        "wk": nc.dram_tensor("wk", [D, HD], BF16, kind="ExternalInput").ap(),
        "wv": nc.dram_tensor("wv", [D, HD], BF16, kind="ExternalInput").ap(),
        "wo": nc.dram_tensor("wo", [2 * HD, D], BF16, kind="ExternalInput").ap(),
        "tabs": nc.dram_tensor("tabs", [P, 4, S], BF16, kind="ExternalInput").ap(),
        "glog": nc.dram_tensor("glog", [P, NJB], F32, kind="ExternalInput").ap(),
        "trimask": nc.dram_tensor("trimask", [P, P], BF16, kind="ExternalInput").ap(),
        "outp": nc.dram_tensor("outp", [S, D], BF16, kind="ExternalOutput").ap(),
    }
    with tile.TileContext(nc) as tc:
        _body(tc, io)
    nc.compile()
    _NC_CACHE = nc
    return nc


def make_in_maps(hidden_states, attention_mask, cos, sin, gate, Wq, Wk, Wv, Wo):
    import ml_dtypes
    bf16 = ml_dtypes.bfloat16
    X = np.asarray(hidden_states, np.float32).reshape(S, D)
    xt = np.ascontiguousarray(X.T.astype(bf16))
    cosT = np.ascontiguousarray(np.asarray(cos, np.float32).reshape(S, HD).T)
    sinT = np.ascontiguousarray(np.asarray(sin, np.float32).reshape(S, HD).T)
    sinTs = np.concatenate([-sinT[: HD // 2], sinT[HD // 2 :]], axis=0)
    sc = np.float32(1.0 / math.sqrt(HD))
    tabs = np.ascontiguousarray(
        np.stack([cosT * sc, sinTs * sc, cosT, sinTs], axis=1).astype(bf16)
    )
    g = np.asarray(gate, np.float32).reshape(S) + np.float32(1e-8)
    glog = np.ascontiguousarray(np.log(g).reshape(NJB, P).T.astype(np.float32))
    jj = np.arange(P)[:, None]
    cc = np.arange(P)[None, :]
    trimask = np.ascontiguousarray((jj <= cc).astype(bf16))

    Wq = np.asarray(Wq, np.float32)
    Wk = np.asarray(Wk, np.float32)
    Wv = np.asarray(Wv, np.float32)
    Wo = np.asarray(Wo, np.float32)

    in_maps = []
    for c in range(8):
        g128 = c // 2
        in_maps.append(
            {
                "xt": xt,
                "wq": np.ascontiguousarray(Wq[:, c * 256 : (c + 1) * 256].astype(bf16)),
                "wk": np.ascontiguousarray(Wk[:, g128 * HD : (g128 + 1) * HD].astype(bf16)),
                "wv": np.ascontiguousarray(Wv[:, g128 * HD : (g128 + 1) * HD].astype(bf16)),
                "wo": np.ascontiguousarray(Wo[c * 256 : (c + 1) * 256, :].astype(bf16)),
                "tabs": tabs,
                "glog": glog,
                "trimask": trimask,
            }
        )
    return in_maps


def kernel(hidden_states, attention_mask, cos, sin, gate, Wq, Wk, Wv, Wo,
           **kwargs):
    nc = build_nc()
    in_maps = make_in_maps(
        hidden_states, attention_mask, cos, sin, gate, Wq, Wk, Wv, Wo
    )
    res = run_bass_kernel_spmd(nc, in_maps, core_ids=list(range(8)), **kwargs)
    acc = res.results[0]["outp"].astype(np.float32)
    for c in range(1, 8):
        acc = acc + res.results[c]["outp"].astype(np.float32)
    out = acc.reshape(1, S, D)
    if kwargs:
        return out, res
    return out
